# revision 1
# baseline (speedup 1.0000x reference)
"""GAT (2-layer, 3-head) forward on 8 Trainium2 NeuronCores.

Sharding: nodes split 8 ways; each core owns 12544 padded destination nodes
and all their incoming edges (1D graph partition per the spec hint). A
channel-major node table (h | a_src | a_dst, 15 ch) is replicated into SBUF
as 4 quarters x 2 copies across the 8 GPSIMD 16-partition groups; per-edge
features stream out via ap_gather with per-group index streams laid out in
dst-canonical order with K=8 slots per (dst, group) (A/B copy balancing;
rare per-(dst,quarter) overflow rows are folded back with a second small
gather). Edge softmax runs densely on DVE/ACT over the slot grid; the
slot-window reduction uses an avg-pool (the num/den ratio is scale
invariant); cross-group partial sums combine with one PE matmul. Three NEFF
launches: (A) table build (x @ W1aug on PE), (B) edge layer 1 + layer-2
table build, (C) edge layer 2 + head-mean + log_softmax. Tables are
all-gathered between launches through the host.
"""
import sys
import types

sys.path.insert(0, "/opt/trn_rl_repo")
import numpy as np

N_NODES = 100000
IN_DIM = 256
HID = 3
HEADS = 3
NCLS = 3
NEG = 0.2
EPS = 1e-16

NQ = 4
QREAL = 25000
QN = 25088
NPAD = NQ * QN          # 100352
NCORE = 8
CN = NPAD // NCORE      # 12544
K = 8
DCHUNK = 224
NCHUNK = CN // DCHUNK   # 56
RPAD = CN + 2 * DCHUNK  # 12992
GCHUNK = RPAD // DCHUNK  # 57
SLOTS = RPAD * K
SCHUNK = DCHUNK * K     # 1792
SENT = QREAL
ZCOL = RPAD - 1
CH = 15
FCH = 8
BIG_NEG = -30000.0

LAST_STATS = {}


def _install_ntff_hook_module():
    if "antenv.axon_hooks" in sys.modules:
        return
    mod = types.ModuleType("antenv.axon_hooks")
    state = {"hook": None, "tried": False}

    def set_axon_ntff_profile_hook(hook):
        state["hook"] = hook

    def get_axon_ntff_profile_hook():
        if state["hook"] is None and not state["tried"]:
            state["tried"] = True
            try:
                from trn_agent_boot.trn_boot import _ntff_profile_via_ctypes

                state["hook"] = _ntff_profile_via_ctypes("/opt/axon/libaxon_pjrt.so")
            except Exception:
                state["hook"] = None
        return state["hook"]

    mod.set_axon_ntff_profile_hook = set_axon_ntff_profile_hook
    mod.get_axon_ntff_profile_hook = get_axon_ntff_profile_hook
    sys.modules["antenv.axon_hooks"] = mod


_install_ntff_hook_module()

import concourse.bass as bass
import concourse.mybir as mybir
import concourse.tile as tile
from concourse.bass_utils import run_bass_kernel_spmd
from concourse.library_overlay import lower_extended_insts
from concourse import library_config

F32 = mybir.dt.float32
I16 = mybir.dt.int16
ALU = mybir.AluOpType
ACT = mybir.ActivationFunctionType


def _split_wide_waits(nc):
    """Walrus here caps sync-wait commands per instruction; hoist excess waits
    onto preceding same-engine nofuse NOPs (engines execute in order)."""
    for fn in nc.m.functions:
        for bb in fn.blocks:
            new_insts = []
            for inst in bb.instructions:
                keep = 0 if isinstance(inst, mybir.InstDrain) else 1
                si = inst.sync_info
                if si is not None and si.on_wait is not None and len(si.on_wait) > keep:
                    waits = list(si.on_wait)
                    head, rest = (waits[:-keep], waits[-keep:]) if keep else (waits, [])
                    while head:
                        chunk, head = head[:1], head[1:]
                        nop = mybir.InstNoOp(name=f"I-{nc.next_id()}", ins=[], outs=[])
                        nop.engine = inst.engine
                        nop.bass_nofuse = True
                        nop.sync_info = mybir.SyncInfo(on_wait=chunk, on_update=[])
                        nc.register_instruction(nop, overwrite=True)
                        new_insts.append(nop)
                    inst.sync_info = mybir.SyncInfo(
                        on_wait=rest, on_update=list(si.on_update or [])
                    )
                new_insts.append(inst)
            bb.instructions.clear()
            for i in new_insts:
                bb.add_instruction(i)


def _run(nc, in_maps, trace=False):
    lower_extended_insts(nc)
    _split_wide_waits(nc)
    return run_bass_kernel_spmd(nc, in_maps, core_ids=list(range(NCORE)), trace=trace)


# ---------------------------------------------------------------- launch A
def _build_phase_a():
    nc = bass.Bass("TRN2")
    xT_d = nc.dram_tensor("xT", [IN_DIM, CN], F32, kind="ExternalInput")
    w1_d = nc.dram_tensor("w1", [IN_DIM, HEADS * HID], F32, kind="ExternalInput")
    w1t_d = nc.dram_tensor("w1t", [HEADS * HID, IN_DIM], F32, kind="ExternalInput")
    attw1_d = nc.dram_tensor("attw1", [HEADS * HID, 6], F32, kind="ExternalInput")
    tab_d = nc.dram_tensor("tab", [CH, CN], F32, kind="ExternalOutput")

    with tile.TileContext(nc) as tc:
        with (
            tc.tile_pool(name="const", bufs=1) as cpool,
            tc.tile_pool(name="io", bufs=3) as iopool,
            tc.tile_pool(name="ps", bufs=2, space="PSUM") as pspool,
        ):
            w1aug = cpool.tile([128, 2 * CH], F32)
            w1t = cpool.tile([HEADS * HID, IN_DIM], F32)
            attw1 = cpool.tile([HEADS * HID, 6], F32)
            nc.sync.dma_start(w1t[:], w1t_d[:])
            nc.sync.dma_start(attw1[:], attw1_d[:])
            for k in range(2):
                nc.sync.dma_start(
                    w1aug[:, CH * k:CH * k + 9], w1_d[128 * k:128 * (k + 1), :]
                )
                vps = pspool.tile([128, 6], F32, tag="vps")
                nc.tensor.matmul(
                    out=vps[:],
                    lhsT=w1t[:, 128 * k:128 * (k + 1)],
                    rhs=attw1[:],
                    start=True,
                    stop=True,
                )
                nc.vector.tensor_copy(out=w1aug[:, CH * k + 9:CH * k + 15], in_=vps[:])
            for c in range(NCHUNK):
                cols = slice(DCHUNK * c, DCHUNK * (c + 1))
                ps = pspool.tile([CH, DCHUNK], F32, tag="ps")
                for k in range(2):
                    xc = iopool.tile([128, DCHUNK], F32, tag="xc")
                    nc.sync.dma_start(xc[:], xT_d[128 * k:128 * (k + 1), cols])
                    nc.tensor.matmul(
                        out=ps[:],
                        lhsT=w1aug[:, CH * k:CH * (k + 1)],
                        rhs=xc[:],
                        start=(k == 0),
                        stop=(k == 1),
                    )
                ob = iopool.tile([CH, DCHUNK], F32, tag="ob")
                nc.vector.tensor_copy(out=ob[:], in_=ps[:])
                nc.sync.dma_start(tab_d[:, cols], ob[:])
    return nc


# ---------------------------------------------------------------- launch B/C
def _build_edge(final):
    nc = bass.Bass("TRN2")
    tab_d = nc.dram_tensor("tabf", [CH, NPAD], F32, kind="ExternalInput")
    idx_d = nc.dram_tensor("idxs", [128, SLOTS // 16], I16, kind="ExternalInput")
    ov_d = nc.dram_tensor("ovidx", [128, CN // 16], I16, kind="ExternalInput")
    adrep_d = nc.dram_tensor("adrep", [24, RPAD], F32, kind="ExternalInput")
    lhsn_d = nc.dram_tensor("lhsn", [128, 9], F32, kind="ExternalInput")
    lhsd_d = nc.dram_tensor("lhsd", [128, 9], F32, kind="ExternalInput")
    bias_d = nc.dram_tensor("biasv", [9, 1], F32, kind="ExternalInput")
    if final:
        meanw_d = nc.dram_tensor("meanw", [9, NCLS], F32, kind="ExternalInput")
        ones3_d = nc.dram_tensor("ones3", [NCLS, 1], F32, kind="ExternalInput")
        ones1_d = nc.dram_tensor("ones1", [1, NCLS], F32, kind="ExternalInput")
        out_d = nc.dram_tensor("outp", [NCLS, CN], F32, kind="ExternalOutput")
    else:
        w2t_d = nc.dram_tensor("w2t", [9, 9], F32, kind="ExternalInput")
        w2_d = nc.dram_tensor("w2", [9, 9], F32, kind="ExternalInput")
        attw2_d = nc.dram_tensor("attw2", [9, 6], F32, kind="ExternalInput")
        tab2_d = nc.dram_tensor("tab2", [CH, CN], F32, kind="ExternalOutput")

    with tile.TileContext(nc) as tc:
        with (
            tc.tile_pool(name="big", bufs=1) as bigpool,
            tc.tile_pool(name="io", bufs=2) as iopool,
            tc.tile_pool(name="gp", bufs=3) as gpool,
            tc.tile_pool(name="sm", bufs=8) as smpool,
            tc.tile_pool(name="ps", bufs=2, space="PSUM") as pspool,
            tc.tile_pool(name="psf", bufs=1, space="PSUM") as psfpool,
        ):
            table = bigpool.tile([128, QN], F32)
            partials = bigpool.tile([128, RPAD], F32)
            nc.vector.memset(partials[:], 0.0)
            for g in range(8):
                q = g % 4
                nc.sync.dma_start(
                    table[16 * g:16 * g + CH, :], tab_d[:, QN * q:QN * (q + 1)]
                )
            lhsn = bigpool.tile([128, 9], F32)
            nc.sync.dma_start(lhsn[:], lhsn_d[:])
            lhsd = bigpool.tile([128, 9], F32)
            nc.sync.dma_start(lhsd[:], lhsd_d[:])
            biasv = bigpool.tile([9, 1], F32)
            nc.sync.dma_start(biasv[:], bias_d[:])
            if final:
                meanw = bigpool.tile([9, NCLS], F32)
                ones3 = bigpool.tile([NCLS, 1], F32)
                ones1 = bigpool.tile([1, NCLS], F32)
                nc.sync.dma_start(meanw[:], meanw_d[:])
                nc.sync.dma_start(ones3[:], ones3_d[:])
                nc.sync.dma_start(ones1[:], ones1_d[:])
            else:
                w2aug = bigpool.tile([9, CH], F32)
                w2t = smpool.tile([9, 9], F32, tag="sm")
                attw2 = smpool.tile([9, 6], F32, tag="sm")
                nc.sync.dma_start(w2t[:], w2t_d[:])
                nc.sync.dma_start(attw2[:], attw2_d[:])
                nc.sync.dma_start(w2aug[:, 0:9], w2_d[:])
                v2ps = psfpool.tile([9, 6], F32, tag="v2")
                nc.tensor.matmul(
                    out=v2ps[:], lhsT=w2t[:], rhs=attw2[:], start=True, stop=True
                )
                nc.vector.tensor_copy(out=w2aug[:, 9:15], in_=v2ps[:])

            tab_in = table[:].rearrange("p (n d) -> p n d", d=1)
            w9 = bigpool.tile([128, SCHUNK], F32)
            nc.vector.memset(w9[:], 1.0)
            nc.gpsimd.load_library(library_config.ap_gather)

            # ---- gather + per-slot softmax weights + messages ----
            for c in range(GCHUNK):
                scol = slice(SCHUNK // 16 * c, SCHUNK // 16 * (c + 1))
                dcol = slice(DCHUNK * c, DCHUNK * (c + 1))
                idxc = iopool.tile([128, SCHUNK // 16], I16, tag="idxc")
                nc.sync.dma_start(idxc[:], idx_d[:, scol])
                g_t = gpool.tile([128, SCHUNK], F32, tag="g")
                nc.gpsimd.ap_gather(
                    out_ap=g_t[:].rearrange("p (n d) -> p n d", d=1),
                    in_ap=tab_in,
                    idxs_ap=idxc[:],
                    channels=128,
                    num_elems=QN,
                    d=1,
                    num_idxs=SCHUNK,
                )
                a24 = iopool.tile([24, DCHUNK], F32, tag="a24")
                nc.sync.dma_start(a24[:], adrep_d[:, dcol])
                wt = iopool.tile([24, SCHUNK], F32, tag="wt")
                for g in range(8):
                    nc.sync.dma_start(
                        wt[3 * g:3 * g + 3, :], g_t[16 * g + 9:16 * g + 12, :]
                    )
                wt3 = wt[:].rearrange("p (n j) -> p n j", j=K)
                nc.vector.tensor_tensor(
                    out=wt3, in0=wt3,
                    in1=a24[:].to_broadcast([24, DCHUNK, K]), op=ALU.add,
                )
                # leaky relu: (x * 0.2) max x, then exp
                nc.vector.scalar_tensor_tensor(
                    out=wt[:], in0=wt[:], scalar=NEG, in1=wt[:],
                    op0=ALU.mult, op1=ALU.max,
                )
                nc.scalar.activation(out=wt[:], in_=wt[:], func=ACT.Exp)
                for h in range(3):
                    nc.sync.dma_start(g_t[9 + h::16, :], wt[h::3, :])
                    for ch3 in range(3):
                        nc.sync.dma_start(w9[3 * h + ch3::16, :], wt[h::3, :])
                nc.vector.tensor_tensor(
                    out=g_t[:], in0=g_t[:], in1=w9[:], op=ALU.mult
                )
                nc.vector.tensor_reduce(
                    out=partials[:, dcol],
                    in_=g_t[:].rearrange("p (n j) -> p n j", j=K),
                    axis=mybir.AxisListType.X,
                    op=ALU.add,
                )

            # ---- overflow fold + cross-group combine + per-node math ----
            par_in = partials[:].rearrange("p (n d) -> p n d", d=1)
            for c in range(NCHUNK):
                dcol = slice(DCHUNK * c, DCHUNK * (c + 1))
                ovc = iopool.tile([128, DCHUNK // 16], I16, tag="ovc")
                nc.sync.dma_start(
                    ovc[:], ov_d[:, DCHUNK // 16 * c:DCHUNK // 16 * (c + 1)]
                )
                foldt = iopool.tile([128, DCHUNK], F32, tag="fold")
                nc.gpsimd.ap_gather(
                    out_ap=foldt[:].rearrange("p (n d) -> p n d", d=1),
                    in_ap=par_in,
                    idxs_ap=ovc[:],
                    channels=128,
                    num_elems=RPAD,
                    d=1,
                    num_idxs=DCHUNK,
                )
                fold = foldt[:, :]
                ndn_ps = pspool.tile([9, DCHUNK], F32, tag="ndn")
                ndd_ps = pspool.tile([9, DCHUNK], F32, tag="ndd")
                nc.tensor.matmul(
                    out=ndn_ps[:], lhsT=lhsn[:], rhs=partials[:, dcol],
                    start=True, stop=False,
                )
                nc.tensor.matmul(
                    out=ndn_ps[:], lhsT=lhsn[:], rhs=fold, start=False, stop=True
                )
                nc.tensor.matmul(
                    out=ndd_ps[:], lhsT=lhsd[:], rhs=partials[:, dcol],
                    start=True, stop=False,
                )
                nc.tensor.matmul(
                    out=ndd_ps[:], lhsT=lhsd[:], rhs=fold, start=False, stop=True
                )
                ndn = smpool.tile([9, DCHUNK], F32, tag="sm")
                nc.vector.tensor_copy(out=ndn[:], in_=ndn_ps[:])
                rden9 = smpool.tile([9, DCHUNK], F32, tag="sm")
                nc.vector.tensor_scalar_add(
                    out=rden9[:], in0=ndd_ps[:], scalar1=EPS
                )
                nc.vector.reciprocal(out=rden9[:], in_=rden9[:])
                hagg = smpool.tile([9, DCHUNK], F32, tag="sm")
                nc.vector.tensor_tensor(
                    out=hagg[:], in0=ndn[:], in1=rden9[:], op=ALU.mult
                )
                if not final:
                    nc.vector.tensor_tensor(
                        out=hagg[:], in0=hagg[:],
                        in1=biasv[:].to_broadcast([9, DCHUNK]), op=ALU.add,
                    )
                    t1 = smpool.tile([9, DCHUNK], F32, tag="sm")
                    nc.vector.tensor_scalar_min(out=t1[:], in0=hagg[:], scalar1=0.0)
                    nc.scalar.activation(out=t1[:], in_=t1[:], func=ACT.Exp)
                    # elu = relu(x) + exp(min(x,0)) - 1
                    nc.vector.tensor_scalar_max(out=hagg[:], in0=hagg[:], scalar1=0.0)
                    nc.vector.tensor_tensor(
                        out=hagg[:], in0=hagg[:], in1=t1[:], op=ALU.add
                    )
                    nc.vector.tensor_scalar_add(out=hagg[:], in0=hagg[:], scalar1=-1.0)
                    t2ps = psfpool.tile([CH, DCHUNK], F32, tag="t2")
                    nc.tensor.matmul(
                        out=t2ps[:], lhsT=w2aug[:], rhs=hagg[:], start=True, stop=True
                    )
                    t2sb = smpool.tile([CH, DCHUNK], F32, tag="sm")
                    nc.vector.tensor_copy(out=t2sb[:], in_=t2ps[:])
                    nc.sync.dma_start(tab2_d[:, dcol], t2sb[:])
                else:
                    zps = psfpool.tile([NCLS, DCHUNK], F32, tag="z")
                    nc.tensor.matmul(
                        out=zps[:], lhsT=meanw[:], rhs=hagg[:], start=True, stop=True
                    )
                    z = smpool.tile([NCLS, DCHUNK], F32, tag="sm")
                    nc.vector.tensor_copy(out=z[:], in_=zps[:])
                    nc.vector.tensor_tensor(
                        out=z[:], in0=z[:],
                        in1=biasv[0:3, :].to_broadcast([NCLS, DCHUNK]), op=ALU.add,
                    )
                    ez = smpool.tile([NCLS, DCHUNK], F32, tag="sm")
                    nc.scalar.activation(out=ez[:], in_=z[:], func=ACT.Exp)
                    sps = psfpool.tile([1, DCHUNK], F32, tag="s")
                    nc.tensor.matmul(
                        out=sps[:], lhsT=ones3[:], rhs=ez[:], start=True, stop=True
                    )
                    s = smpool.tile([1, DCHUNK], F32, tag="sm")
                    nc.vector.tensor_copy(out=s[:], in_=sps[:])
                    nc.scalar.activation(out=s[:], in_=s[:], func=ACT.Ln)
                    l3ps = psfpool.tile([NCLS, DCHUNK], F32, tag="l3")
                    nc.tensor.matmul(
                        out=l3ps[:], lhsT=ones1[:], rhs=s[:], start=True, stop=True
                    )
                    l3 = smpool.tile([NCLS, DCHUNK], F32, tag="sm")
                    nc.vector.tensor_copy(out=l3[:], in_=l3ps[:])
                    zm = smpool.tile([NCLS, DCHUNK], F32, tag="sm")
                    nc.vector.tensor_tensor(
                        out=zm[:], in0=z[:], in1=l3[:], op=ALU.subtract
                    )
                    nc.sync.dma_start(out_d[:, dcol], zm[:])
    return nc


# ---------------------------------------------------------------- host side
def _relabel(n):
    q = n // QREAL
    return q * QN + n % QREAL


def _wrap_chunked(stream, chunk):
    """[G, S] streams -> [16G, S//16] ap_gather idx layout, wrapped per chunk."""
    g, s = stream.shape
    nch = s // chunk
    w = stream.reshape(g, nch, chunk // 16, 16)
    w = w.transpose(0, 3, 1, 2)
    return np.ascontiguousarray(w.reshape(g * 16, s // 16))


def _pack_edges(src, dst):
    srcN = _relabel(src.astype(np.int64))
    dstN = _relabel(dst.astype(np.int64))
    core = dstN // CN
    dloc = dstN % CN
    q = srcN // QN
    sloc = (srcN % QN).astype(np.int16)

    key = (core * CN + dloc) * 4 + q
    order = np.argsort(key, kind="stable")
    ks = key[order]
    grp_start = np.r_[0, np.flatnonzero(np.diff(ks)) + 1]
    sizes = np.diff(np.r_[grp_start, len(ks)])
    rank = np.arange(len(ks)) - np.repeat(grp_start, sizes)

    co, dl, qo, sl = core[order], dloc[order], q[order], sloc[order]

    streams = np.full((NCORE, 8, SLOTS), SENT, dtype=np.int16)
    ovidx = np.full((NCORE, CN), ZCOL, dtype=np.int16)
    ovdst = [[] for _ in range(NCORE)]

    main = rank < 16
    gmain = qo[main] + 4 * (rank[main] & 1)
    pos = dl[main] * K + (rank[main] >> 1)
    streams[co[main], gmain, pos] = sl[main]

    for i in np.flatnonzero(~main):
        c, d, qq, s_, r = co[i], dl[i], qo[i], sl[i], rank[i]
        if ovidx[c, d] == ZCOL:
            row = CN + len(ovdst[c])
            assert row < RPAD - 1, "overflow area exhausted"
            ovidx[c, d] = row
            ovdst[c].append(int(d))
        rr = r - 16
        assert rr < 16, "per-(dst,quarter) capacity exceeded"
        g = qq + 4 * (rr & 1)
        streams[c, g, int(ovidx[c, d]) * K + (rr >> 1)] = s_
    return streams, ovidx, ovdst


def kernel(x, edge_index, W1, att_src1, att_dst1, b1, W2, att_src2, att_dst2, b2):
    import os as _os
    import time as _time

    x = np.asarray(x, np.float32)
    W1 = np.asarray(W1, np.float32)
    W2 = np.asarray(W2, np.float32)
    b1v = np.asarray(b1, np.float32)
    b2v = np.asarray(b2, np.float32)

    loops = np.arange(N_NODES, dtype=np.int64)
    src = np.concatenate([np.asarray(edge_index[0], np.int64), loops])
    dst = np.concatenate([np.asarray(edge_index[1], np.int64), loops])
    streams, ovidx, ovdst = _pack_edges(src, dst)

    xP = np.zeros((NPAD, IN_DIM), np.float32)
    xP[_relabel(np.arange(N_NODES))] = x
    xT = np.ascontiguousarray(xP.T)

    def attw(att_s, att_d):
        a = np.zeros((HEADS * HID, 6), np.float32)
        for h in range(HEADS):
            for cc in range(3):
                a[3 * h + cc, h] = np.asarray(att_s, np.float32)[h, cc]
                a[3 * h + cc, 3 + h] = np.asarray(att_d, np.float32)[h, cc]
        return a

    attw1 = attw(att_src1, att_dst1)
    attw2 = attw(att_src2, att_dst2)

    lhsn = np.zeros((128, 9), np.float32)
    lhsd = np.zeros((128, 9), np.float32)
    for p in range(128):
        j = p % 16
        if j < 9:
            lhsn[p, j] = 1.0
        elif j < 12:
            for cc in range(3):
                lhsd[p, 3 * (j - 9) + cc] = 1.0
    meanw = np.zeros((9, NCLS), np.float32)
    for h in range(HEADS):
        for cc in range(NCLS):
            meanw[3 * h + cc, cc] = 1.0 / 3.0
    ones3 = np.ones((NCLS, 1), np.float32)
    ones1 = np.ones((1, NCLS), np.float32)
    b1m = b1v.reshape(9, 1).copy()
    b2m = np.zeros((9, 1), np.float32)
    b2m[:NCLS, 0] = b2v

    idx_wr = np.stack([_wrap_chunked(streams[c], SCHUNK) for c in range(NCORE)])
    ov_wr = np.stack(
        [
            _wrap_chunked(np.repeat(ovidx[c].reshape(1, CN), 8, axis=0), DCHUNK)
            for c in range(NCORE)
        ]
    )

    def make_adrep(tab):
        out = []
        for c in range(NCORE):
            ad = np.zeros((3, RPAD), np.float32)
            ad[:, :CN] = tab[12:15, CN * c:CN * (c + 1)]
            for i, d in enumerate(ovdst[c]):
                ad[:, CN + i] = tab[12:15, CN * c + d]
            rep = np.zeros((24, RPAD), np.float32)
            for g in range(8):
                rep[3 * g:3 * g + 3, :] = ad
            out.append(rep)
        return out

    trace = bool(int(_os.environ.get("KERNEL_TRACE", "0")))
    stats = {}
    t0 = _time.time()

    ncA = _build_phase_a()
    in_maps = [
        {
            "xT": np.ascontiguousarray(xT[:, CN * c:CN * (c + 1)]),
            "w1": W1,
            "w1t": np.ascontiguousarray(W1.T),
            "attw1": attw1,
        }
        for c in range(NCORE)
    ]
    resA = _run(ncA, in_maps, trace=trace)
    stats["A_ns"] = resA.exec_time_ns
    tab1 = np.concatenate([resA.results[c]["tab"] for c in range(NCORE)], axis=1)
    padmask = np.zeros(NPAD, bool)
    for qq in range(NQ):
        padmask[QN * qq + QREAL:QN * (qq + 1)] = True
    tab1[9:12, padmask] = BIG_NEG

    ncB = _build_edge(final=False)
    adreps = make_adrep(tab1)
    in_maps = [
        {
            "tabf": tab1,
            "idxs": idx_wr[c],
            "ovidx": ov_wr[c],
            "adrep": adreps[c],
            "lhsn": lhsn,
            "lhsd": lhsd,
            "biasv": b1m,
            "w2t": np.ascontiguousarray(W2.T),
            "w2": W2,
            "attw2": attw2,
        }
        for c in range(NCORE)
    ]
    resB = _run(ncB, in_maps, trace=trace)
    stats["B_ns"] = resB.exec_time_ns
    tab2 = np.concatenate([resB.results[c]["tab2"] for c in range(NCORE)], axis=1)
    tab2[9:12, padmask] = BIG_NEG

    ncC = _build_edge(final=True)
    adreps = make_adrep(tab2)
    in_maps = [
        {
            "tabf": tab2,
            "idxs": idx_wr[c],
            "ovidx": ov_wr[c],
            "adrep": adreps[c],
            "lhsn": lhsn,
            "lhsd": lhsd,
            "biasv": b2m,
            "meanw": meanw,
            "ones3": ones3,
            "ones1": ones1,
        }
        for c in range(NCORE)
    ]
    resC = _run(ncC, in_maps, trace=trace)
    stats["C_ns"] = resC.exec_time_ns
    outT = np.concatenate([resC.results[c]["outp"] for c in range(NCORE)], axis=1)
    stats["wall_s"] = _time.time() - t0

    out = outT.T[_relabel(np.arange(N_NODES))]
    LAST_STATS.clear()
    LAST_STATS.update(stats)
    return np.ascontiguousarray(out, dtype=np.float32)



# revision 6
# speedup vs baseline: 1.1755x; 1.1755x over previous
"""GAT (2-layer, 3-head) forward on 8 Trainium2 NeuronCores.

Sharding: nodes split 8 ways; each core owns 12544 padded destination nodes
and all their incoming edges (1D graph partition per the spec hint).

Edge layers use an interleaved fp16 node table in SBUF: per 16-partition
GPSIMD group g (quarter q=g%4, copy g//4), partition p holds the pair
(h_p[n], asrc_head(p)[n]) for every node n of the quarter; partitions 9-11
hold (1.0, asrc_h[n]) so that a single multiply+windowed-reduce produces
both the message numerator rows (0-8) and the softmax denominator rows
(9-11) of the partials tile. One ap_gather (d=2) per 224-dst chunk pulls
everything per edge; per-slot softmax weights are computed in place with
one DVE add (a_dst broadcast), ACT leaky-relu + exp on the odd (logit)
lanes, then DVE multiply (even *= odd) and a K=8 windowed reduce. The
cross-group/cross-copy combine, per-(dst,quarter) overflow fold (second
small gather), bias/ELU/log-softmax tail run batched over 448-dst chunks.
Three NEFF launches: (A) x @ W1aug table build, (B) edge layer 1 +
layer-2 table build, (C) edge layer 2 + head-mean + log_softmax. Tables
are all-gathered between launches through the host.
"""
import sys
import types

sys.path.insert(0, "/opt/trn_rl_repo")
import numpy as np

N_NODES = 100000
IN_DIM = 256
HID = 3
HEADS = 3
NCLS = 3
NEG = 0.2
EPS = 1e-16

NQ = 4
QREAL = 25000
QN = 25088
NPAD = NQ * QN          # 100352
NCORE = 8
CN = NPAD // NCORE      # 12544
K = 8
DCHUNK = 224
RPAD = CN + 2 * DCHUNK  # 12992
GCHUNK = RPAD // DCHUNK  # 58
SLOTS = RPAD * K
SCHUNK = DCHUNK * K     # 1792
TCHUNK = 448            # tail combine chunk
TN = CN // TCHUNK       # 28
SENT = QREAL
ZCOL = RPAD - 1
CH = 15
BIG_NEG = -30000.0
HEADMAP = (0, 0, 0, 1, 1, 1, 2, 2, 2)

LAST_STATS = {}


def _install_ntff_hook_module():
    if "antenv.axon_hooks" in sys.modules:
        return
    mod = types.ModuleType("antenv.axon_hooks")
    state = {"hook": None, "tried": False}

    def set_axon_ntff_profile_hook(hook):
        state["hook"] = hook

    def get_axon_ntff_profile_hook():
        if state["hook"] is None and not state["tried"]:
            state["tried"] = True
            try:
                from trn_agent_boot.trn_boot import _ntff_profile_via_ctypes

                state["hook"] = _ntff_profile_via_ctypes("/opt/axon/libaxon_pjrt.so")
            except Exception:
                state["hook"] = None
        return state["hook"]

    mod.set_axon_ntff_profile_hook = set_axon_ntff_profile_hook
    mod.get_axon_ntff_profile_hook = get_axon_ntff_profile_hook
    sys.modules["antenv.axon_hooks"] = mod


_install_ntff_hook_module()

import concourse.bass as bass
import concourse.mybir as mybir
import concourse.tile as tile
from concourse.bass_utils import run_bass_kernel_spmd
from concourse.library_overlay import lower_extended_insts
from concourse import library_config

F32 = mybir.dt.float32
F16 = mybir.dt.float16
I16 = mybir.dt.int16
ALU = mybir.AluOpType
ACT = mybir.ActivationFunctionType


def _split_wide_waits(nc):
    """Walrus here caps sync-wait commands per instruction; hoist excess waits
    onto preceding same-engine nofuse NOPs (engines execute in order)."""
    for fn in nc.m.functions:
        for bb in fn.blocks:
            new_insts = []
            for inst in bb.instructions:
                keep = 0 if isinstance(inst, mybir.InstDrain) else 1
                si = inst.sync_info
                if si is not None and si.on_wait is not None and len(si.on_wait) > keep:
                    waits = list(si.on_wait)
                    head, rest = (waits[:-keep], waits[-keep:]) if keep else (waits, [])
                    while head:
                        chunk, head = head[:1], head[1:]
                        nop = mybir.InstNoOp(name=f"I-{nc.next_id()}", ins=[], outs=[])
                        nop.engine = inst.engine
                        nop.bass_nofuse = True
                        nop.sync_info = mybir.SyncInfo(on_wait=chunk, on_update=[])
                        nc.register_instruction(nop, overwrite=True)
                        new_insts.append(nop)
                    inst.sync_info = mybir.SyncInfo(
                        on_wait=rest, on_update=list(si.on_update or [])
                    )
                new_insts.append(inst)
            bb.instructions.clear()
            for i in new_insts:
                bb.add_instruction(i)


def _run(nc, in_maps, trace=False):
    lower_extended_insts(nc)
    _split_wide_waits(nc)
    return run_bass_kernel_spmd(nc, in_maps, core_ids=list(range(NCORE)), trace=trace)


# ---------------------------------------------------------------- launch A
def _build_phase_a():
    nc = bass.Bass("TRN2")
    xT_d = nc.dram_tensor("xT", [IN_DIM, CN], F32, kind="ExternalInput")
    w1_d = nc.dram_tensor("w1", [IN_DIM, HEADS * HID], F32, kind="ExternalInput")
    w1t_d = nc.dram_tensor("w1t", [HEADS * HID, IN_DIM], F32, kind="ExternalInput")
    attw1_d = nc.dram_tensor("attw1", [HEADS * HID, 6], F32, kind="ExternalInput")
    tab_d = nc.dram_tensor("tab", [CH, CN], F32, kind="ExternalOutput")

    DA = 448
    NA = CN // DA
    with tile.TileContext(nc) as tc:
        with (
            tc.tile_pool(name="const", bufs=1) as cpool,
            tc.tile_pool(name="io", bufs=4) as iopool,
            tc.tile_pool(name="ps", bufs=4, space="PSUM") as pspool,
        ):
            w1aug = cpool.tile([128, 2 * CH], F32)
            w1t = cpool.tile([HEADS * HID, IN_DIM], F32)
            attw1 = cpool.tile([HEADS * HID, 6], F32)
            nc.sync.dma_start(w1t[:], w1t_d[:])
            nc.sync.dma_start(attw1[:], attw1_d[:])
            for k in range(2):
                nc.sync.dma_start(
                    w1aug[:, CH * k:CH * k + 9], w1_d[128 * k:128 * (k + 1), :]
                )
                vps = pspool.tile([128, 6], F32, tag="vps")
                nc.tensor.matmul(
                    out=vps[:],
                    lhsT=w1t[:, 128 * k:128 * (k + 1)],
                    rhs=attw1[:],
                    start=True,
                    stop=True,
                )
                nc.vector.tensor_copy(out=w1aug[:, CH * k + 9:CH * k + 15], in_=vps[:])
            for c in range(NA):
                cols = slice(DA * c, DA * (c + 1))
                ps = pspool.tile([CH, DA], F32, tag="ps")
                for k in range(2):
                    xc = iopool.tile([128, DA], F32, tag="xc")
                    nc.sync.dma_start(xc[:], xT_d[128 * k:128 * (k + 1), cols])
                    nc.tensor.matmul(
                        out=ps[:],
                        lhsT=w1aug[:, CH * k:CH * (k + 1)],
                        rhs=xc[:],
                        start=(k == 0),
                        stop=(k == 1),
                    )
                ob = iopool.tile([CH, DA], F32, tag="ob")
                nc.vector.tensor_copy(out=ob[:], in_=ps[:])
                nc.sync.dma_start(tab_d[:, cols], ob[:])
    return nc


# ---------------------------------------------------------------- launch B/C
def _build_edge(final):
    nc = bass.Bass("TRN2")
    tabi_d = nc.dram_tensor("tabi", [128, 2 * QN], F16, kind="ExternalInput")
    idx_d = nc.dram_tensor("idxs", [128, SLOTS // 16], I16, kind="ExternalInput")
    ov_d = nc.dram_tensor("ovidx", [128, CN // 16], I16, kind="ExternalInput")
    arep_d = nc.dram_tensor("arep", [128, RPAD], F16, kind="ExternalInput")
    lhsn_d = nc.dram_tensor("lhsn", [128, 9], F32, kind="ExternalInput")
    lhsd_d = nc.dram_tensor("lhsd", [128, 9], F32, kind="ExternalInput")
    bias_d = nc.dram_tensor("biasv", [9, 1], F32, kind="ExternalInput")
    if final:
        meanw_d = nc.dram_tensor("meanw", [9, NCLS], F32, kind="ExternalInput")
        ones3_d = nc.dram_tensor("ones3", [NCLS, 1], F32, kind="ExternalInput")
        ones1_d = nc.dram_tensor("ones1", [1, NCLS], F32, kind="ExternalInput")
        out_d = nc.dram_tensor("outp", [NCLS, CN], F32, kind="ExternalOutput")
    else:
        w2t_d = nc.dram_tensor("w2t", [9, 9], F32, kind="ExternalInput")
        w2_d = nc.dram_tensor("w2", [9, 9], F32, kind="ExternalInput")
        attw2_d = nc.dram_tensor("attw2", [9, 6], F32, kind="ExternalInput")
        tab2_d = nc.dram_tensor("tab2", [CH, CN], F32, kind="ExternalOutput")

    with tile.TileContext(nc) as tc:
        with (
            tc.tile_pool(name="big", bufs=1) as bigpool,
            tc.tile_pool(name="io", bufs=3) as iopool,
            tc.tile_pool(name="gp", bufs=3) as gpool,
            tc.tile_pool(name="sm", bufs=8) as smpool,
            tc.tile_pool(name="ps", bufs=2, space="PSUM") as pspool,
            tc.tile_pool(name="psf", bufs=1, space="PSUM") as psfpool,
        ):
            table = bigpool.tile([128, 2 * QN], F16)
            partials = bigpool.tile([128, RPAD], F32)
            for g in range(8):
                nc.sync.dma_start(
                    table[16 * g:16 * (g + 1), :], tabi_d[16 * g:16 * (g + 1), :]
                )
            lhsn = bigpool.tile([128, 9], F32)
            nc.sync.dma_start(lhsn[:], lhsn_d[:])
            lhsd = bigpool.tile([128, 9], F32)
            nc.sync.dma_start(lhsd[:], lhsd_d[:])
            biasv = bigpool.tile([9, 1], F32)
            nc.sync.dma_start(biasv[:], bias_d[:])
            if final:
                meanw = bigpool.tile([9, NCLS], F32)
                ones3 = bigpool.tile([NCLS, 1], F32)
                ones1 = bigpool.tile([1, NCLS], F32)
                nc.sync.dma_start(meanw[:], meanw_d[:])
                nc.sync.dma_start(ones3[:], ones3_d[:])
                nc.sync.dma_start(ones1[:], ones1_d[:])
            else:
                w2aug = bigpool.tile([9, CH], F32)
                w2t = smpool.tile([9, 9], F32, tag="sm")
                attw2 = smpool.tile([9, 6], F32, tag="sm")
                nc.sync.dma_start(w2t[:], w2t_d[:])
                nc.sync.dma_start(attw2[:], attw2_d[:])
                nc.sync.dma_start(w2aug[:, 0:9], w2_d[:])
                v2ps = psfpool.tile([9, 6], F32, tag="t2")
                nc.tensor.matmul(
                    out=v2ps[:], lhsT=w2t[:], rhs=attw2[:], start=True, stop=True
                )
                nc.vector.tensor_copy(out=w2aug[:, 9:15], in_=v2ps[:])

            cneg3 = bigpool.tile([128, 1], F32)
            nc.vector.memset(cneg3[:], -3.0)
            tab_in = table[:].rearrange("p (n d) -> p n d", d=2)
            nc.gpsimd.load_library(library_config.ap_gather)

            # ---- gather + in-place edge softmax weights + messages ----
            for c in range(GCHUNK):
                scol = slice(SCHUNK // 16 * c, SCHUNK // 16 * (c + 1))
                dcol = slice(DCHUNK * c, DCHUNK * (c + 1))
                idxc = iopool.tile([128, SCHUNK // 16], I16, tag="idxc")
                nc.sync.dma_start(idxc[:], idx_d[:, scol])
                g_t = gpool.tile([128, 2 * SCHUNK], F16, tag="g")
                nc.gpsimd.ap_gather(
                    out_ap=g_t[:].rearrange("p (n d) -> p n d", d=2),
                    in_ap=tab_in,
                    idxs_ap=idxc[:],
                    channels=128,
                    num_elems=QN,
                    d=2,
                    num_idxs=SCHUNK,
                )
                av = iopool.tile([128, DCHUNK], F16, tag="av")
                nc.sync.dma_start(av[:], arep_d[:, dcol])
                v4 = g_t[:].rearrange("p (d k two) -> p d k two", k=K, two=2)
                odd = v4[:, :, :, 1]
                even = v4[:, :, :, 0]
                nc.vector.tensor_tensor(
                    out=odd, in0=odd,
                    in1=av[:].to_broadcast([128, DCHUNK, K]), op=ALU.add,
                )
                nc.scalar.activation(out=odd, in_=odd, func=ACT.Lrelu, alpha=NEG)
                # bias -3: scales all softmax weights by e^-3 (ratio-invariant)
                # to keep h*w inside fp16 range.
                nc.scalar.activation(out=odd, in_=odd, func=ACT.Exp, bias=cneg3[:])
                nc.vector.tensor_tensor(out=even, in0=even, in1=odd, op=ALU.mult)
                nc.vector.tensor_reduce(
                    out=partials[:, dcol],
                    in_=even,
                    axis=mybir.AxisListType.X,
                    op=ALU.add,
                )

            # ---- overflow fold + cross-group combine + per-node math ----
            par_in = partials[:].rearrange("p (n d) -> p n d", d=1)
            for c in range(TN):
                dcol = slice(TCHUNK * c, TCHUNK * (c + 1))
                ovc = iopool.tile([128, TCHUNK // 16], I16, tag="ovc")
                nc.sync.dma_start(
                    ovc[:], ov_d[:, TCHUNK // 16 * c:TCHUNK // 16 * (c + 1)]
                )
                foldt = iopool.tile([128, TCHUNK], F32, tag="fold")
                nc.gpsimd.ap_gather(
                    out_ap=foldt[:].rearrange("p (n d) -> p n d", d=1),
                    in_ap=par_in,
                    idxs_ap=ovc[:],
                    channels=128,
                    num_elems=RPAD,
                    d=1,
                    num_idxs=TCHUNK,
                )
                ndn_ps = pspool.tile([9, TCHUNK], F32, tag="ndn")
                ndd_ps = pspool.tile([9, TCHUNK], F32, tag="ndd")
                nc.tensor.matmul(
                    out=ndn_ps[:], lhsT=lhsn[:], rhs=partials[:, dcol],
                    start=True, stop=False,
                )
                nc.tensor.matmul(
                    out=ndn_ps[:], lhsT=lhsn[:], rhs=foldt[:], start=False, stop=True
                )
                nc.tensor.matmul(
                    out=ndd_ps[:], lhsT=lhsd[:], rhs=partials[:, dcol],
                    start=True, stop=False,
                )
                nc.tensor.matmul(
                    out=ndd_ps[:], lhsT=lhsd[:], rhs=foldt[:], start=False, stop=True
                )
                rden9 = smpool.tile([9, TCHUNK], F32, tag="sm")
                nc.vector.tensor_scalar_add(
                    out=rden9[:], in0=ndd_ps[:], scalar1=EPS
                )
                nc.vector.reciprocal(out=rden9[:], in_=rden9[:])
                hagg = smpool.tile([9, TCHUNK], F32, tag="sm")
                nc.vector.tensor_tensor(
                    out=hagg[:], in0=ndn_ps[:], in1=rden9[:], op=ALU.mult
                )
                if not final:
                    # elu(x) + b == max(x,0) + exp(min(x,0)) + (b-1); host passes b-1
                    t1 = smpool.tile([9, TCHUNK], F32, tag="sm")
                    nc.vector.tensor_scalar_min(out=t1[:], in0=hagg[:], scalar1=0.0)
                    nc.scalar.activation(out=t1[:], in_=t1[:], func=ACT.Exp)
                    nc.vector.tensor_scalar_max(out=hagg[:], in0=hagg[:], scalar1=0.0)
                    nc.vector.tensor_tensor(
                        out=hagg[:], in0=hagg[:], in1=t1[:], op=ALU.add
                    )
                    nc.vector.tensor_tensor(
                        out=hagg[:], in0=hagg[:],
                        in1=biasv[:].to_broadcast([9, TCHUNK]), op=ALU.add,
                    )
                    t2ps = psfpool.tile([CH, TCHUNK], F32, tag="t2")
                    nc.tensor.matmul(
                        out=t2ps[:], lhsT=w2aug[:], rhs=hagg[:], start=True, stop=True
                    )
                    t2sb = smpool.tile([CH, TCHUNK], F32, tag="sm")
                    nc.vector.tensor_copy(out=t2sb[:], in_=t2ps[:])
                    nc.sync.dma_start(tab2_d[:, dcol], t2sb[:])
                else:
                    zps = psfpool.tile([NCLS, TCHUNK], F32, tag="t2")
                    nc.tensor.matmul(
                        out=zps[:], lhsT=meanw[:], rhs=hagg[:], start=True, stop=True
                    )
                    z = smpool.tile([NCLS, TCHUNK], F32, tag="sm")
                    nc.vector.tensor_tensor(
                        out=z[:], in0=zps[:],
                        in1=biasv[0:3, :].to_broadcast([NCLS, TCHUNK]), op=ALU.add,
                    )
                    ez = smpool.tile([NCLS, TCHUNK], F32, tag="sm")
                    nc.scalar.activation(out=ez[:], in_=z[:], func=ACT.Exp)
                    sps = psfpool.tile([1, TCHUNK], F32, tag="s")
                    nc.tensor.matmul(
                        out=sps[:], lhsT=ones3[:], rhs=ez[:], start=True, stop=True
                    )
                    s = smpool.tile([1, TCHUNK], F32, tag="sm")
                    nc.vector.tensor_copy(out=s[:], in_=sps[:])
                    nc.scalar.activation(out=s[:], in_=s[:], func=ACT.Ln)
                    l3ps = psfpool.tile([NCLS, TCHUNK], F32, tag="l3")
                    nc.tensor.matmul(
                        out=l3ps[:], lhsT=ones1[:], rhs=s[:], start=True, stop=True
                    )
                    zm = smpool.tile([NCLS, TCHUNK], F32, tag="sm")
                    nc.vector.tensor_tensor(
                        out=zm[:], in0=z[:], in1=l3ps[:], op=ALU.subtract
                    )
                    nc.sync.dma_start(out_d[:, dcol], zm[:])
    return nc


# ---------------------------------------------------------------- host side
def _relabel(n):
    q = n // QREAL
    return q * QN + n % QREAL


def _wrap_chunked(stream, chunk):
    """[G, S] streams -> [16G, S//16] ap_gather idx layout, wrapped per chunk."""
    g, s = stream.shape
    nch = s // chunk
    w = stream.reshape(g, nch, chunk // 16, 16)
    w = w.transpose(0, 3, 1, 2)
    return np.ascontiguousarray(w.reshape(g * 16, s // 16))


def _pack_edges(src, dst):
    srcN = _relabel(src.astype(np.int64))
    dstN = _relabel(dst.astype(np.int64))
    core = dstN // CN
    dloc = dstN % CN
    q = srcN // QN
    sloc = (srcN % QN).astype(np.int16)

    key = (core * CN + dloc) * 4 + q
    order = np.argsort(key, kind="stable")
    ks = key[order]
    grp_start = np.r_[0, np.flatnonzero(np.diff(ks)) + 1]
    sizes = np.diff(np.r_[grp_start, len(ks)])
    rank = np.arange(len(ks)) - np.repeat(grp_start, sizes)

    co, dl, qo, sl = core[order], dloc[order], q[order], sloc[order]

    streams = np.full((NCORE, 8, SLOTS), SENT, dtype=np.int16)
    ovidx = np.full((NCORE, CN), ZCOL, dtype=np.int16)
    ovdst = [[] for _ in range(NCORE)]

    main = rank < 16
    gmain = qo[main] + 4 * (rank[main] & 1)
    pos = dl[main] * K + (rank[main] >> 1)
    streams[co[main], gmain, pos] = sl[main]

    for i in np.flatnonzero(~main):
        c, d, qq, s_, r = co[i], dl[i], qo[i], sl[i], rank[i]
        if ovidx[c, d] == ZCOL:
            row = CN + len(ovdst[c])
            assert row < RPAD - 1, "overflow area exhausted"
            ovidx[c, d] = row
            ovdst[c].append(int(d))
        rr = r - 16
        assert rr < 16, "per-(dst,quarter) capacity exceeded"
        g = qq + 4 * (rr & 1)
        streams[c, g, int(ovidx[c, d]) * K + (rr >> 1)] = s_
    return streams, ovidx, ovdst


def _make_timg(tab):
    """tab [15, NPAD] f32 -> interleaved fp16 table image [128, 2*QN]."""
    timg = np.zeros((128, QN, 2), np.float16)
    for g in range(8):
        qq = g % 4
        sl = tab[:, QN * qq:QN * (qq + 1)]
        timg[16 * g:16 * g + 9, :, 0] = sl[0:9]
        timg[16 * g + 9:16 * g + 12, :, 0] = 1.0
        for p in range(9):
            timg[16 * g + p, :, 1] = sl[9 + HEADMAP[p]]
        for j in range(3):
            timg[16 * g + 9 + j, :, 1] = sl[9 + j]
    return np.ascontiguousarray(timg.reshape(128, 2 * QN))


def kernel(x, edge_index, W1, att_src1, att_dst1, b1, W2, att_src2, att_dst2, b2):
    import os as _os
    import time as _time

    x = np.asarray(x, np.float32)
    W1 = np.asarray(W1, np.float32)
    W2 = np.asarray(W2, np.float32)
    b1v = np.asarray(b1, np.float32)
    b2v = np.asarray(b2, np.float32)

    loops = np.arange(N_NODES, dtype=np.int64)
    src = np.concatenate([np.asarray(edge_index[0], np.int64), loops])
    dst = np.concatenate([np.asarray(edge_index[1], np.int64), loops])
    streams, ovidx, ovdst = _pack_edges(src, dst)

    xP = np.zeros((NPAD, IN_DIM), np.float32)
    xP[_relabel(np.arange(N_NODES))] = x
    xT = np.ascontiguousarray(xP.T)

    def attw(att_s, att_d):
        a = np.zeros((HEADS * HID, 6), np.float32)
        for h in range(HEADS):
            for cc in range(3):
                a[3 * h + cc, h] = np.asarray(att_s, np.float32)[h, cc]
                a[3 * h + cc, 3 + h] = np.asarray(att_d, np.float32)[h, cc]
        return a

    attw1 = attw(att_src1, att_dst1)
    attw2 = attw(att_src2, att_dst2)

    lhsn = np.zeros((128, 9), np.float32)
    lhsd = np.zeros((128, 9), np.float32)
    for p in range(128):
        j = p % 16
        if j < 9:
            lhsn[p, j] = 1.0
        elif j < 12:
            for cc in range(3):
                lhsd[p, 3 * (j - 9) + cc] = 1.0
    meanw = np.zeros((9, NCLS), np.float32)
    for h in range(HEADS):
        for cc in range(NCLS):
            meanw[3 * h + cc, cc] = 1.0 / 3.0
    ones3 = np.ones((NCLS, 1), np.float32)
    ones1 = np.ones((1, NCLS), np.float32)
    b1m = b1v.reshape(9, 1) - 1.0  # folded ELU's -1
    b2m = np.zeros((9, 1), np.float32)
    b2m[:NCLS, 0] = b2v

    idx_wr = np.stack([_wrap_chunked(streams[c], SCHUNK) for c in range(NCORE)])
    ov_wr = np.stack(
        [
            _wrap_chunked(np.repeat(ovidx[c].reshape(1, CN), 8, axis=0), TCHUNK)
            for c in range(NCORE)
        ]
    )

    def make_arep(tab):
        out = []
        for c in range(NCORE):
            ad3 = np.zeros((3, RPAD), np.float32)
            ad3[:, :CN] = tab[12:15, CN * c:CN * (c + 1)]
            for i, d in enumerate(ovdst[c]):
                ad3[:, CN + i] = tab[12:15, CN * c + d]
            a = np.zeros((128, RPAD), np.float16)
            for g in range(8):
                for p in range(9):
                    a[16 * g + p] = ad3[HEADMAP[p]]
                for j in range(3):
                    a[16 * g + 9 + j] = ad3[j]
            out.append(a)
        return out

    trace = bool(int(_os.environ.get("KERNEL_TRACE", "0")))
    stats = {}
    t0 = _time.time()

    ncA = _build_phase_a()
    in_maps = [
        {
            "xT": np.ascontiguousarray(xT[:, CN * c:CN * (c + 1)]),
            "w1": W1,
            "w1t": np.ascontiguousarray(W1.T),
            "attw1": attw1,
        }
        for c in range(NCORE)
    ]
    resA = _run(ncA, in_maps, trace=trace)
    stats["A_ns"] = resA.exec_time_ns
    tab1 = np.concatenate([resA.results[c]["tab"] for c in range(NCORE)], axis=1)
    padmask = np.zeros(NPAD, bool)
    for qq in range(NQ):
        padmask[QN * qq + QREAL:QN * (qq + 1)] = True
    tab1[9:12, padmask] = BIG_NEG

    ncB = _build_edge(final=False)
    timg1 = _make_timg(tab1)
    areps = make_arep(tab1)
    in_maps = [
        {
            "tabi": timg1,
            "idxs": idx_wr[c],
            "ovidx": ov_wr[c],
            "arep": areps[c],
            "lhsn": lhsn,
            "lhsd": lhsd,
            "biasv": b1m,
            "w2t": np.ascontiguousarray(W2.T),
            "w2": W2,
            "attw2": attw2,
        }
        for c in range(NCORE)
    ]
    resB = _run(ncB, in_maps, trace=trace)
    stats["B_ns"] = resB.exec_time_ns
    tab2 = np.concatenate([resB.results[c]["tab2"] for c in range(NCORE)], axis=1)
    tab2[9:12, padmask] = BIG_NEG

    ncC = _build_edge(final=True)
    timg2 = _make_timg(tab2)
    areps = make_arep(tab2)
    in_maps = [
        {
            "tabi": timg2,
            "idxs": idx_wr[c],
            "ovidx": ov_wr[c],
            "arep": areps[c],
            "lhsn": lhsn,
            "lhsd": lhsd,
            "biasv": b2m,
            "meanw": meanw,
            "ones3": ones3,
            "ones1": ones1,
        }
        for c in range(NCORE)
    ]
    resC = _run(ncC, in_maps, trace=trace)
    stats["C_ns"] = resC.exec_time_ns
    outT = np.concatenate([resC.results[c]["outp"] for c in range(NCORE)], axis=1)
    stats["wall_s"] = _time.time() - t0

    out = outT.T[_relabel(np.arange(N_NODES))]
    LAST_STATS.clear()
    LAST_STATS.update(stats)
    return np.ascontiguousarray(out, dtype=np.float32)


# revision 22
# speedup vs baseline: 1.2810x; 1.0897x over previous
"""GAT (2-layer, 3-head) forward on 8 Trainium2 NeuronCores.

Sharding: nodes split 8 ways; each core owns 12544 padded destination nodes
and all their incoming edges (1D graph partition per the spec hint).

Edge layers use an interleaved fp16 node table in SBUF: per 16-partition
GPSIMD group g (quarter q=g%4, copy g//4), partition p holds the pair
(h_p[n], asrc_head(p)[n]) for every node n of the quarter; partitions 9-11
hold (1.0, asrc_h[n]) so that a single multiply+windowed-reduce produces
both the message numerator rows (0-8) and the softmax denominator rows
(9-11) of the partials tile. One ap_gather (d=2) per 224-dst chunk pulls
everything per edge; per-slot softmax weights are computed in place with
one DVE add (a_dst broadcast), DVE leaky-relu and ACT exp (bias -6 keeps
h*w in fp16 range; attention is ratio-invariant to it) on the odd (logit)
lanes, then DVE multiply (even *= odd) and a K=7 windowed reduce. The
cross-group/cross-copy combine, per-(dst,quarter) overflow fold (second
small gather), bias/ELU/log-softmax tail run batched over 448-dst chunks.
Three NEFF launches: (A) x @ W1aug table build, (B) edge layer 1 +
layer-2 table build, (C) edge layer 2 + head-mean + log_softmax. Tables
are all-gathered between launches through the host.
"""
import sys
import types

sys.path.insert(0, "/opt/trn_rl_repo")
import numpy as np

N_NODES = 100000
IN_DIM = 256
HID = 3
HEADS = 3
NCLS = 3
NEG = 0.2
EPS = 1e-16

NQ = 4
QREAL = 25000
QN = 25088
NPAD = NQ * QN          # 100352
NCORE = 8
CN = NPAD // NCORE      # 12544
K = 7
DCHUNK = 224
RPAD = CN + 5 * DCHUNK  # 13664
GCHUNK = RPAD // DCHUNK  # 61
NMAIN = CN // DCHUNK    # 56 main chunks; 56..60 are overflow columns
SLOTS = RPAD * K
SCHUNK = DCHUNK * K     # 1568
TCHUNK = 448            # tail combine chunk
TN = CN // TCHUNK       # 28
SENT = QREAL
ZCOL = RPAD - 1
CH = 15
BIG_NEG = -30000.0
HEADMAP = (0, 0, 0, 1, 1, 1, 2, 2, 2)

LAST_STATS = {}


def _install_ntff_hook_module():
    if "antenv.axon_hooks" in sys.modules:
        return
    mod = types.ModuleType("antenv.axon_hooks")
    state = {"hook": None, "tried": False}

    def set_axon_ntff_profile_hook(hook):
        state["hook"] = hook

    def get_axon_ntff_profile_hook():
        if state["hook"] is None and not state["tried"]:
            state["tried"] = True
            try:
                from trn_agent_boot.trn_boot import _ntff_profile_via_ctypes

                state["hook"] = _ntff_profile_via_ctypes("/opt/axon/libaxon_pjrt.so")
            except Exception:
                state["hook"] = None
        return state["hook"]

    mod.set_axon_ntff_profile_hook = set_axon_ntff_profile_hook
    mod.get_axon_ntff_profile_hook = get_axon_ntff_profile_hook
    sys.modules["antenv.axon_hooks"] = mod


_install_ntff_hook_module()

import concourse.bass as bass
import concourse.mybir as mybir
import concourse.tile as tile
from concourse.bass_utils import run_bass_kernel_spmd
from concourse.library_overlay import lower_extended_insts
from concourse import library_config

F32 = mybir.dt.float32
F16 = mybir.dt.float16
I16 = mybir.dt.int16
ALU = mybir.AluOpType
ACT = mybir.ActivationFunctionType


def _split_wide_waits(nc):
    """Walrus here caps sync-wait commands per instruction; hoist excess waits
    onto preceding same-engine nofuse NOPs (engines execute in order)."""
    for fn in nc.m.functions:
        for bb in fn.blocks:
            new_insts = []
            for inst in bb.instructions:
                keep = 0 if isinstance(inst, mybir.InstDrain) else 1
                si = inst.sync_info
                if si is not None and si.on_wait is not None and len(si.on_wait) > keep:
                    waits = list(si.on_wait)
                    head, rest = (waits[:-keep], waits[-keep:]) if keep else (waits, [])
                    while head:
                        chunk, head = head[:1], head[1:]
                        nop = mybir.InstNoOp(name=f"I-{nc.next_id()}", ins=[], outs=[])
                        nop.engine = inst.engine
                        nop.bass_nofuse = True
                        nop.sync_info = mybir.SyncInfo(on_wait=chunk, on_update=[])
                        nc.register_instruction(nop, overwrite=True)
                        new_insts.append(nop)
                    inst.sync_info = mybir.SyncInfo(
                        on_wait=rest, on_update=list(si.on_update or [])
                    )
                new_insts.append(inst)
            bb.instructions.clear()
            for i in new_insts:
                bb.add_instruction(i)


def _run(nc, in_maps, trace=False):
    lower_extended_insts(nc)
    _split_wide_waits(nc)
    return run_bass_kernel_spmd(nc, in_maps, core_ids=list(range(NCORE)), trace=trace)


# ---------------------------------------------------------------- launch A
def _build_phase_a():
    nc = bass.Bass("TRN2")
    xT_d = nc.dram_tensor("xT", [IN_DIM, CN], F32, kind="ExternalInput")
    w1_d = nc.dram_tensor("w1", [IN_DIM, HEADS * HID], F32, kind="ExternalInput")
    w1t_d = nc.dram_tensor("w1t", [HEADS * HID, IN_DIM], F32, kind="ExternalInput")
    attw1_d = nc.dram_tensor("attw1", [HEADS * HID, 6], F32, kind="ExternalInput")
    tab_d = nc.dram_tensor("tab", [CH, CN], F32, kind="ExternalOutput")

    DA = 448
    NA = CN // DA
    with tile.TileContext(nc) as tc:
        with (
            tc.tile_pool(name="const", bufs=1) as cpool,
            tc.tile_pool(name="io", bufs=4) as iopool,
            tc.tile_pool(name="ps", bufs=4, space="PSUM") as pspool,
        ):
            w1aug = cpool.tile([128, 2 * CH], F32)
            w1t = cpool.tile([HEADS * HID, IN_DIM], F32)
            attw1 = cpool.tile([HEADS * HID, 6], F32)
            nc.sync.dma_start(w1t[:], w1t_d[:])
            nc.sync.dma_start(attw1[:], attw1_d[:])
            for k in range(2):
                nc.sync.dma_start(
                    w1aug[:, CH * k:CH * k + 9], w1_d[128 * k:128 * (k + 1), :]
                )
                vps = pspool.tile([128, 6], F32, tag="vps")
                nc.tensor.matmul(
                    out=vps[:],
                    lhsT=w1t[:, 128 * k:128 * (k + 1)],
                    rhs=attw1[:],
                    start=True,
                    stop=True,
                )
                nc.vector.tensor_copy(out=w1aug[:, CH * k + 9:CH * k + 15], in_=vps[:])
            for c in range(NA):
                cols = slice(DA * c, DA * (c + 1))
                ps = pspool.tile([CH, DA], F32, tag="ps")
                for k in range(2):
                    xc = iopool.tile([128, DA], F32, tag="xc")
                    nc.sync.dma_start(xc[:], xT_d[128 * k:128 * (k + 1), cols])
                    nc.tensor.matmul(
                        out=ps[:],
                        lhsT=w1aug[:, CH * k:CH * (k + 1)],
                        rhs=xc[:],
                        start=(k == 0),
                        stop=(k == 1),
                    )
                ob = iopool.tile([CH, DA], F32, tag="ob")
                nc.vector.tensor_copy(out=ob[:], in_=ps[:])
                nc.sync.dma_start(tab_d[:, cols], ob[:])
    return nc


# ---------------------------------------------------------------- launch B/C
def _build_edge(final):
    nc = bass.Bass("TRN2")
    tabi_d = nc.dram_tensor("tabi", [128, 2 * QN], F16, kind="ExternalInput")
    idx_d = nc.dram_tensor("idxs", [128, SLOTS // 16], I16, kind="ExternalInput")
    ov_d = nc.dram_tensor("ovidx", [128, CN // 16], I16, kind="ExternalInput")
    arep_d = nc.dram_tensor("arep", [128, RPAD], F16, kind="ExternalInput")
    lhsn_d = nc.dram_tensor("lhsn", [128, 9], F32, kind="ExternalInput")
    lhsd_d = nc.dram_tensor("lhsd", [128, 9], F32, kind="ExternalInput")
    bias_d = nc.dram_tensor("biasv", [9, 1], F32, kind="ExternalInput")
    if final:
        meanw_d = nc.dram_tensor("meanw", [9, NCLS], F32, kind="ExternalInput")
        ones3_d = nc.dram_tensor("ones3", [NCLS, 1], F32, kind="ExternalInput")
        ones1_d = nc.dram_tensor("ones1", [1, NCLS], F32, kind="ExternalInput")
        out_d = nc.dram_tensor("outp", [NCLS, CN], F32, kind="ExternalOutput")
    else:
        w2t_d = nc.dram_tensor("w2t", [9, 9], F32, kind="ExternalInput")
        w2_d = nc.dram_tensor("w2", [9, 9], F32, kind="ExternalInput")
        attw2_d = nc.dram_tensor("attw2", [9, 6], F32, kind="ExternalInput")
        tab2_d = nc.dram_tensor("tab2", [CH, CN], F32, kind="ExternalOutput")

    with tile.TileContext(nc) as tc:
        with (
            tc.tile_pool(name="big", bufs=1) as bigpool,
            tc.tile_pool(name="io", bufs=3) as iopool,
            tc.tile_pool(name="gp", bufs=3) as gpool,
            tc.tile_pool(name="sm", bufs=8) as smpool,
            tc.tile_pool(name="ps", bufs=2, space="PSUM") as pspool,
            tc.tile_pool(name="psf", bufs=1, space="PSUM") as psfpool,
        ):
            table = bigpool.tile([128, 2 * QN], F16)
            partials = bigpool.tile([128, RPAD], F32)
            for g in range(8):
                nc.sync.dma_start(
                    table[16 * g:16 * (g + 1), :], tabi_d[16 * g:16 * (g + 1), :]
                )
            lhsn = bigpool.tile([128, 9], F32)
            nc.sync.dma_start(lhsn[:], lhsn_d[:])
            lhsd = bigpool.tile([128, 9], F32)
            nc.sync.dma_start(lhsd[:], lhsd_d[:])
            biasv = bigpool.tile([9, 1], F32)
            nc.sync.dma_start(biasv[:], bias_d[:])
            if final:
                meanw = bigpool.tile([9, NCLS], F32)
                ones3 = bigpool.tile([NCLS, 1], F32)
                ones1 = bigpool.tile([1, NCLS], F32)
                nc.sync.dma_start(meanw[:], meanw_d[:])
                nc.sync.dma_start(ones3[:], ones3_d[:])
                nc.sync.dma_start(ones1[:], ones1_d[:])
            else:
                w2aug = bigpool.tile([9, CH], F32)
                w2t = smpool.tile([9, 9], F32, tag="sm")
                attw2 = smpool.tile([9, 6], F32, tag="sm")
                nc.sync.dma_start(w2t[:], w2t_d[:])
                nc.sync.dma_start(attw2[:], attw2_d[:])
                nc.sync.dma_start(w2aug[:, 0:9], w2_d[:])
                v2ps = psfpool.tile([9, 6], F32, tag="t2")
                nc.tensor.matmul(
                    out=v2ps[:], lhsT=w2t[:], rhs=attw2[:], start=True, stop=True
                )
                nc.vector.tensor_copy(out=w2aug[:, 9:15], in_=v2ps[:])

            cneg3 = bigpool.tile([128, 1], F32)
            nc.vector.memset(cneg3[:], -6.0)
            tab_in = table[:].rearrange("p (n d) -> p n d", d=2)
            nc.gpsimd.load_library(library_config.ap_gather)

            # ---- gather + in-place edge softmax weights + messages ----
            def main_chunk(c):
                scol = slice(SCHUNK // 16 * c, SCHUNK // 16 * (c + 1))
                dcol = slice(DCHUNK * c, DCHUNK * (c + 1))
                idxc = iopool.tile([128, SCHUNK // 16], I16, tag="idxc")
                nc.sync.dma_start(idxc[:], idx_d[:, scol])
                g_t = gpool.tile([128, 2 * SCHUNK], F16, tag="g")
                nc.gpsimd.ap_gather(
                    out_ap=g_t[:].rearrange("p (n d) -> p n d", d=2),
                    in_ap=tab_in,
                    idxs_ap=idxc[:],
                    channels=128,
                    num_elems=QN,
                    d=2,
                    num_idxs=SCHUNK,
                )
                av = iopool.tile([128, DCHUNK], F16, tag="av")
                nc.sync.dma_start(av[:], arep_d[:, dcol])
                v4 = g_t[:].rearrange("p (d k two) -> p d k two", k=K, two=2)
                odd = v4[:, :, :, 1]
                even = v4[:, :, :, 0]
                nc.vector.tensor_tensor(
                    out=odd, in0=odd,
                    in1=av[:].to_broadcast([128, DCHUNK, K]), op=ALU.add,
                )
                # leaky relu on DVE (HW ACT Lrelu ignores alpha): (x*0.2) max x
                nc.vector.scalar_tensor_tensor(
                    out=odd, in0=odd, scalar=NEG, in1=odd,
                    op0=ALU.mult, op1=ALU.max,
                )
                # bias -6: scales all softmax weights by e^-6 (ratio-invariant)
                # to keep h*w inside fp16 range.
                nc.scalar.activation(out=odd, in_=odd, func=ACT.Exp, bias=cneg3[:])
                nc.vector.tensor_tensor(out=even, in0=even, in1=odd, op=ALU.mult)
                nc.vector.tensor_reduce(
                    out=partials[:, dcol],
                    in_=even,
                    axis=mybir.AxisListType.X,
                    op=ALU.add,
                )

            for c in range(GCHUNK):
                main_chunk(c)

            # ---- overflow fold + cross-group combine + per-node math,
            #      interleaved with the gather stream ----
            par_in = partials[:].rearrange("p (n d) -> p n d", d=1)

            def tail_chunk(c):
                dcol = slice(TCHUNK * c, TCHUNK * (c + 1))
                ovc = iopool.tile([128, TCHUNK // 16], I16, tag="ovc")
                nc.sync.dma_start(
                    ovc[:], ov_d[:, TCHUNK // 16 * c:TCHUNK // 16 * (c + 1)]
                )
                foldt = iopool.tile([128, TCHUNK], F32, tag="fold")
                nc.gpsimd.ap_gather(
                    out_ap=foldt[:].rearrange("p (n d) -> p n d", d=1),
                    in_ap=par_in,
                    idxs_ap=ovc[:],
                    channels=128,
                    num_elems=RPAD,
                    d=1,
                    num_idxs=TCHUNK,
                )
                ndn_ps = pspool.tile([9, TCHUNK], F32, tag="ndn")
                ndd_ps = pspool.tile([9, TCHUNK], F32, tag="ndd")
                nc.tensor.matmul(
                    out=ndn_ps[:], lhsT=lhsn[:], rhs=partials[:, dcol],
                    start=True, stop=False,
                )
                nc.tensor.matmul(
                    out=ndn_ps[:], lhsT=lhsn[:], rhs=foldt[:], start=False, stop=True
                )
                nc.tensor.matmul(
                    out=ndd_ps[:], lhsT=lhsd[:], rhs=partials[:, dcol],
                    start=True, stop=False,
                )
                nc.tensor.matmul(
                    out=ndd_ps[:], lhsT=lhsd[:], rhs=foldt[:], start=False, stop=True
                )
                rden9 = smpool.tile([9, TCHUNK], F32, tag="sm")
                nc.vector.tensor_scalar_add(
                    out=rden9[:], in0=ndd_ps[:], scalar1=EPS
                )
                nc.vector.reciprocal(out=rden9[:], in_=rden9[:])
                hagg = smpool.tile([9, TCHUNK], F32, tag="sm")
                nc.vector.tensor_tensor(
                    out=hagg[:], in0=ndn_ps[:], in1=rden9[:], op=ALU.mult
                )
                if not final:
                    # elu(x) + b == max(x,0) + exp(min(x,0)) + (b-1); host passes b-1
                    t1 = smpool.tile([9, TCHUNK], F32, tag="sm")
                    nc.vector.tensor_scalar_min(out=t1[:], in0=hagg[:], scalar1=0.0)
                    nc.scalar.activation(out=t1[:], in_=t1[:], func=ACT.Exp)
                    nc.vector.tensor_scalar_max(out=hagg[:], in0=hagg[:], scalar1=0.0)
                    nc.vector.tensor_tensor(
                        out=hagg[:], in0=hagg[:], in1=t1[:], op=ALU.add
                    )
                    nc.vector.tensor_tensor(
                        out=hagg[:], in0=hagg[:],
                        in1=biasv[:].to_broadcast([9, TCHUNK]), op=ALU.add,
                    )
                    t2ps = psfpool.tile([CH, TCHUNK], F32, tag="t2")
                    nc.tensor.matmul(
                        out=t2ps[:], lhsT=w2aug[:], rhs=hagg[:], start=True, stop=True
                    )
                    t2sb = smpool.tile([CH, TCHUNK], F32, tag="sm")
                    nc.vector.tensor_copy(out=t2sb[:], in_=t2ps[:])
                    nc.sync.dma_start(tab2_d[:, dcol], t2sb[:])
                else:
                    zps = psfpool.tile([NCLS, TCHUNK], F32, tag="t2")
                    nc.tensor.matmul(
                        out=zps[:], lhsT=meanw[:], rhs=hagg[:], start=True, stop=True
                    )
                    z = smpool.tile([NCLS, TCHUNK], F32, tag="sm")
                    nc.vector.tensor_tensor(
                        out=z[:], in0=zps[:],
                        in1=biasv[0:3, :].to_broadcast([NCLS, TCHUNK]), op=ALU.add,
                    )
                    ez = smpool.tile([NCLS, TCHUNK], F32, tag="sm")
                    nc.scalar.activation(out=ez[:], in_=z[:], func=ACT.Exp)
                    sps = psfpool.tile([1, TCHUNK], F32, tag="s")
                    nc.tensor.matmul(
                        out=sps[:], lhsT=ones3[:], rhs=ez[:], start=True, stop=True
                    )
                    s = smpool.tile([1, TCHUNK], F32, tag="sm")
                    nc.vector.tensor_copy(out=s[:], in_=sps[:])
                    nc.scalar.activation(out=s[:], in_=s[:], func=ACT.Ln)
                    l3ps = psfpool.tile([NCLS, TCHUNK], F32, tag="l3")
                    nc.tensor.matmul(
                        out=l3ps[:], lhsT=ones1[:], rhs=s[:], start=True, stop=True
                    )
                    zm = smpool.tile([NCLS, TCHUNK], F32, tag="sm")
                    nc.vector.tensor_tensor(
                        out=zm[:], in0=z[:], in1=l3ps[:], op=ALU.subtract
                    )
                    nc.sync.dma_start(out_d[:, dcol], zm[:])

            for t in range(TN):
                tail_chunk(t)
    return nc


# ---------------------------------------------------------------- host side
def _relabel(n):
    q = n // QREAL
    return q * QN + n % QREAL


def _wrap_chunked(stream, chunk):
    """[G, S] streams -> [16G, S//16] ap_gather idx layout, wrapped per chunk."""
    g, s = stream.shape
    nch = s // chunk
    w = stream.reshape(g, nch, chunk // 16, 16)
    w = w.transpose(0, 3, 1, 2)
    return np.ascontiguousarray(w.reshape(g * 16, s // 16))


def _pack_edges(src, dst):
    srcN = _relabel(src.astype(np.int64))
    dstN = _relabel(dst.astype(np.int64))
    core = dstN // CN
    dloc = dstN % CN
    q = srcN // QN
    sloc = (srcN % QN).astype(np.int16)

    key = (core * CN + dloc) * 4 + q
    order = np.argsort(key, kind="stable")
    ks = key[order]
    grp_start = np.r_[0, np.flatnonzero(np.diff(ks)) + 1]
    sizes = np.diff(np.r_[grp_start, len(ks)])
    rank = np.arange(len(ks)) - np.repeat(grp_start, sizes)

    co, dl, qo, sl = core[order], dloc[order], q[order], sloc[order]

    streams = np.full((NCORE, 8, SLOTS), SENT, dtype=np.int16)
    ovidx = np.full((NCORE, CN), ZCOL, dtype=np.int16)
    ovdst = [[] for _ in range(NCORE)]

    main = rank < 2 * K
    gmain = qo[main] + 4 * (rank[main] & 1)
    pos = dl[main] * K + (rank[main] >> 1)
    streams[co[main], gmain, pos] = sl[main]

    for i in np.flatnonzero(~main):
        c, d, qq, s_, r = co[i], dl[i], qo[i], sl[i], rank[i]
        if ovidx[c, d] == ZCOL:
            row = CN + len(ovdst[c])
            assert row < RPAD - 1, "overflow area exhausted"
            ovidx[c, d] = row
            ovdst[c].append(int(d))
        rr = r - 2 * K
        assert rr < 2 * K, "per-(dst,quarter) capacity exceeded"
        g = qq + 4 * (rr & 1)
        streams[c, g, int(ovidx[c, d]) * K + (rr >> 1)] = s_
    return streams, ovidx, ovdst


def _make_timg(tab):
    """tab [15, NPAD] f32 -> interleaved fp16 table image [128, 2*QN]."""
    timg = np.zeros((128, QN, 2), np.float16)
    for g in range(8):
        qq = g % 4
        sl = tab[:, QN * qq:QN * (qq + 1)]
        timg[16 * g:16 * g + 9, :, 0] = sl[0:9]
        timg[16 * g + 9:16 * g + 12, :, 0] = 1.0
        for p in range(9):
            timg[16 * g + p, :, 1] = sl[9 + HEADMAP[p]]
        for j in range(3):
            timg[16 * g + 9 + j, :, 1] = sl[9 + j]
    return np.ascontiguousarray(timg.reshape(128, 2 * QN))


def kernel(x, edge_index, W1, att_src1, att_dst1, b1, W2, att_src2, att_dst2, b2):
    import os as _os
    import time as _time

    x = np.asarray(x, np.float32)
    W1 = np.asarray(W1, np.float32)
    W2 = np.asarray(W2, np.float32)
    b1v = np.asarray(b1, np.float32)
    b2v = np.asarray(b2, np.float32)

    loops = np.arange(N_NODES, dtype=np.int64)
    src = np.concatenate([np.asarray(edge_index[0], np.int64), loops])
    dst = np.concatenate([np.asarray(edge_index[1], np.int64), loops])
    streams, ovidx, ovdst = _pack_edges(src, dst)

    xP = np.zeros((NPAD, IN_DIM), np.float32)
    xP[_relabel(np.arange(N_NODES))] = x
    xT = np.ascontiguousarray(xP.T)

    def attw(att_s, att_d):
        a = np.zeros((HEADS * HID, 6), np.float32)
        for h in range(HEADS):
            for cc in range(3):
                a[3 * h + cc, h] = np.asarray(att_s, np.float32)[h, cc]
                a[3 * h + cc, 3 + h] = np.asarray(att_d, np.float32)[h, cc]
        return a

    attw1 = attw(att_src1, att_dst1)
    attw2 = attw(att_src2, att_dst2)

    lhsn = np.zeros((128, 9), np.float32)
    lhsd = np.zeros((128, 9), np.float32)
    for p in range(128):
        j = p % 16
        if j < 9:
            lhsn[p, j] = 1.0
        elif j < 12:
            for cc in range(3):
                lhsd[p, 3 * (j - 9) + cc] = 1.0
    meanw = np.zeros((9, NCLS), np.float32)
    for h in range(HEADS):
        for cc in range(NCLS):
            meanw[3 * h + cc, cc] = 1.0 / 3.0
    ones3 = np.ones((NCLS, 1), np.float32)
    ones1 = np.ones((1, NCLS), np.float32)
    b1m = b1v.reshape(9, 1) - 1.0  # folded ELU's -1
    b2m = np.zeros((9, 1), np.float32)
    b2m[:NCLS, 0] = b2v

    idx_wr = np.stack([_wrap_chunked(streams[c], SCHUNK) for c in range(NCORE)])
    ov_wr = np.stack(
        [
            _wrap_chunked(np.repeat(ovidx[c].reshape(1, CN), 8, axis=0), TCHUNK)
            for c in range(NCORE)
        ]
    )

    def make_arep(tab):
        out = []
        for c in range(NCORE):
            ad3 = np.zeros((3, RPAD), np.float32)
            ad3[:, :CN] = tab[12:15, CN * c:CN * (c + 1)]
            for i, d in enumerate(ovdst[c]):
                ad3[:, CN + i] = tab[12:15, CN * c + d]
            a = np.zeros((128, RPAD), np.float16)
            for g in range(8):
                for p in range(9):
                    a[16 * g + p] = ad3[HEADMAP[p]]
                for j in range(3):
                    a[16 * g + 9 + j] = ad3[j]
            out.append(a)
        return out

    trace = bool(int(_os.environ.get("KERNEL_TRACE", "0")))
    stats = {}
    t0 = _time.time()

    ncA = _build_phase_a()
    in_maps = [
        {
            "xT": np.ascontiguousarray(xT[:, CN * c:CN * (c + 1)]),
            "w1": W1,
            "w1t": np.ascontiguousarray(W1.T),
            "attw1": attw1,
        }
        for c in range(NCORE)
    ]
    resA = _run(ncA, in_maps, trace=trace)
    stats["A_ns"] = resA.exec_time_ns
    tab1 = np.concatenate([resA.results[c]["tab"] for c in range(NCORE)], axis=1)
    padmask = np.zeros(NPAD, bool)
    for qq in range(NQ):
        padmask[QN * qq + QREAL:QN * (qq + 1)] = True
    tab1[9:12, padmask] = BIG_NEG

    ncB = _build_edge(final=False)
    timg1 = _make_timg(tab1)
    areps = make_arep(tab1)
    in_maps = [
        {
            "tabi": timg1,
            "idxs": idx_wr[c],
            "ovidx": ov_wr[c],
            "arep": areps[c],
            "lhsn": lhsn,
            "lhsd": lhsd,
            "biasv": b1m,
            "w2t": np.ascontiguousarray(W2.T),
            "w2": W2,
            "attw2": attw2,
        }
        for c in range(NCORE)
    ]
    resB = _run(ncB, in_maps, trace=trace)
    stats["B_ns"] = resB.exec_time_ns
    tab2 = np.concatenate([resB.results[c]["tab2"] for c in range(NCORE)], axis=1)
    tab2[9:12, padmask] = BIG_NEG

    ncC = _build_edge(final=True)
    timg2 = _make_timg(tab2)
    areps = make_arep(tab2)
    in_maps = [
        {
            "tabi": timg2,
            "idxs": idx_wr[c],
            "ovidx": ov_wr[c],
            "arep": areps[c],
            "lhsn": lhsn,
            "lhsd": lhsd,
            "biasv": b2m,
            "meanw": meanw,
            "ones3": ones3,
            "ones1": ones1,
        }
        for c in range(NCORE)
    ]
    resC = _run(ncC, in_maps, trace=trace)
    stats["C_ns"] = resC.exec_time_ns
    outT = np.concatenate([resC.results[c]["outp"] for c in range(NCORE)], axis=1)
    stats["wall_s"] = _time.time() - t0

    out = outT.T[_relabel(np.arange(N_NODES))]
    LAST_STATS.clear()
    LAST_STATS.update(stats)
    return np.ascontiguousarray(out, dtype=np.float32)
